# revision 6
# baseline (speedup 1.0000x reference)
"""Multi-head attention (b=2, n=2048, d=1024, h=16) on 8 TRN2 NeuronCores.

Sharding: data-parallel over batch (2) x tensor-parallel over head-groups (4).
Core c handles batch c//4, heads 4*(c%4)..4*(c%4)+3 (channel rows 256*(c%4)..).
Column-parallel QKV, row-parallel output projection with an on-device
ReduceScatter over each 4-core batch group; each core emits a 512-token slice
of the final output which the host concatenates.

All host-side prep is layout-only (slicing + transposes): the device receives
x^T and the weight shards pre-transposed so every matmul operand is already in
its natural (contraction-on-partition) layout.
"""

import sys
from contextlib import ExitStack

_TRN_REPO = "/opt/trn_rl_repo"
if _TRN_REPO not in sys.path:
    sys.path.insert(0, _TRN_REPO)

import numpy as np

import concourse.bass as bass
import concourse.bacc as bacc
import concourse.tile as tile
from concourse import mybir

F32 = mybir.dt.float32
F32R = mybir.dt.float32r

# Problem constants (hardcoded per the harness contract)
B = 2          # batch
N = 2048       # tokens
D = 1024       # model dim
H = 16         # heads
HD = D // H    # 64 head dim
N_CORES = 8
GROUPS = [[0, 1, 2, 3], [4, 5, 6, 7]]
HPC = 4        # heads per core
CPC = HPC * HD  # 256 channels per core


def build_program(n=N):
    """Build + compile the SPMD Bacc program. n is parameterizable for sim tests."""
    assert n % 512 == 0
    nj = n // 128          # key tiles
    nih = n // 1024 if n >= 1024 else 1  # i halves
    ihw = min(n, 1024)     # i-half width
    ntok_out = n // 4      # tokens owned per core after ReduceScatter

    nc = bacc.Bacc("TRN2", target_bir_lowering=False, debug=False,
                   num_devices=N_CORES)

    # ---- DRAM I/O (per-core shards, host-prepared) ----
    # inputs feeding matmuls are declared fp32r end-to-end (hw rounds on read)
    xt_d = nc.dram_tensor("xt", [D, n], F32R, kind="ExternalInput").ap()
    wqt_d = nc.dram_tensor("wqt", [D, CPC], F32R, kind="ExternalInput").ap()
    wkt_d = nc.dram_tensor("wkt", [D, CPC], F32R, kind="ExternalInput").ap()
    wvt_d = nc.dram_tensor("wvt", [D, CPC], F32R, kind="ExternalInput").ap()
    wot_d = nc.dram_tensor("wot", [CPC, D], F32R, kind="ExternalInput").ap()
    bo_d = nc.dram_tensor("bob", [128, D], F32, kind="ExternalInput").ap()
    out_d = nc.dram_tensor("out", [ntok_out, D], F32, kind="ExternalOutput").ap()

    partial_d = nc.dram_tensor("partial", [n, D], F32).ap()
    rs_out_d = nc.dram_tensor("rs_out", [ntok_out, D], F32).ap()

    with tile.TileContext(nc) as tc, ExitStack() as octx:
        # persistent pools
        wpool = octx.enter_context(tc.tile_pool(name="wpool", bufs=1))
        qk_pool = octx.enter_context(tc.tile_pool(name="qk", bufs=1))
        v_pool = octx.enter_context(tc.tile_pool(name="vaug", bufs=1))
        o_pool = octx.enter_context(tc.tile_pool(name="opair", bufs=1))
        mm_ps = octx.enter_context(tc.tile_pool(name="mmps", bufs=2, space="PSUM"))
        ot_ps = octx.enter_context(tc.tile_pool(name="otps", bufs=2, space="PSUM"))

        # ---- weights: load + round to fp32r ----
        # wqt/wkt/wvt as [128, 8*256] (c-chunk n at cols n*256..), wot as [128, 2*1024]
        def load_w(name, dram, rows, cols):
            nch = rows // 128
            raw = wpool.tile([128, nch * cols], F32R, tag=name, name=name + "_t")
            nc.sync.dma_start(
                raw[:].rearrange("p (c m) -> p c m", c=nch),
                dram.rearrange("(c p) m -> p c m", p=128))
            return raw[:]

        wqt = load_w("wqt", wqt_d, D, CPC)
        wkt = load_w("wkt", wkt_d, D, CPC)
        wvt = load_w("wvt", wvt_d, D, CPC)
        wot = load_w("wot", wot_d, CPC, D)

        bias_sb = wpool.tile([128, D], F32, tag="bias")
        nc.sync.dma_start(bias_sb[:], bo_d[:])

        # ones column vector for K=1 broadcast matmuls (memset can't write
        # fp32r, so memset f32 then round via DVE copy)
        ones_f = wpool.tile([128, 64], F32, tag="ones_f")
        nc.gpsimd.memset(ones_f[:], 1.0)
        ones1 = wpool.tile([1, 64], F32R, tag="ones1")
        nc.vector.tensor_copy(ones1[:], ones_f[0:1, :])

        # Qt/Kt head-pair tiles [128, n] fp32r: pair p rows 0-63 = head 2p, 64-127 = head 2p+1
        qtp = [qk_pool.tile([128, n], F32R, tag=f"qtp{p}", name=f"qtp{p}") for p in range(2)]
        ktp = [qk_pool.tile([128, n], F32R, tag=f"ktp{p}", name=f"ktp{p}") for p in range(2)]
        # V augmented: per j-tile [128, 4*65], head h at cols 65h..65h+64, ones at 65h+64
        vaug = [v_pool.tile([128, HPC * 65], F32R, tag=f"vaug{j}", name=f"vaug{j}") for j in range(nj)]
        # normalized O^T pairs: pair p rows = heads 2p,2p+1
        opair = [o_pool.tile([128, n], F32R, tag=f"op{p}", name=f"op{p}") for p in range(2)]

        with tc.tile_pool(name="xt", bufs=1) as xt_pool:
            # ---- x^T load: 8 chunk tiles [128, n] fp32r ----
            xt_raw = [xt_pool.tile([128, n], F32R, tag=f"xtr{ch}", name=f"xtr{ch}") for ch in range(8)]
            xt_sb = [t[:] for t in xt_raw]
            for ch in range(8):
                nc.sync.dma_start(xt_raw[ch][:], xt_d[128 * ch:128 * (ch + 1), :])

            # ---- QKV projections ----
            # Q/K per pair p: psum [128, 512] per i-chunk, accumulate 8 c-chunks
            for p in range(2):
                for (wmat, dst) in ((wqt, qtp), (wkt, ktp)):
                    for ic in range(n // 512):
                        ps = mm_ps.tile([128, 1024], F32, tag="mm")
                        for ch in range(8):
                            nc.tensor.matmul(
                                ps[:, 0:512],
                                wmat[:, ch * 256 + p * 128: ch * 256 + p * 128 + 128],
                                xt_sb[ch][:, 512 * ic: 512 * (ic + 1)],
                                start=(ch == 0), stop=(ch == 7))
                        nc.vector.tensor_copy(
                            dst[p][:, 512 * ic: 512 * (ic + 1)], ps[:, 0:512])

            # ---- V (+ones) ----
            for j in range(nj):
                nc.vector.tensor_copy(
                    vaug[j][:].rearrange("p (h m) -> p h m", h=HPC)[:, :, 64:65],
                    ones_f[:].rearrange("p (h m) -> p h m", m=1)[:, 0:HPC, :])
                ps = mm_ps.tile([128, 1024], F32, tag="mm")
                for ch in range(8):
                    nc.tensor.matmul(
                        ps[:, 0:256],
                        xt_sb[ch][:, 128 * j: 128 * (j + 1)],
                        wvt[:, ch * 256: ch * 256 + 256],
                        start=(ch == 0), stop=(ch == 7))
                # strided copy into heads' 65-wide slots (cols 65h..65h+63)
                dst = vaug[j][:].rearrange("p (h m) -> p h m", h=HPC)[:, :, 0:64]
                src = ps[:, 0:256].rearrange("p (h m) -> p h m", h=HPC)
                nc.vector.tensor_copy(dst, src)

        # ---- attention per (head, i-half) ----
        scale = float(HD) ** -0.5
        with tc.tile_pool(name="st", bufs=4) as st_pool, \
             tc.tile_pool(name="nrm", bufs=2) as nrm_pool:
            for h in range(HPC):
                for ih in range(nih):
                    i0 = ih * ihw
                    ot = ot_ps.tile([65, ihw], F32, tag="ot")
                    for j in range(nj):
                        st_ps = mm_ps.tile([128, 1024], F32, tag="mm")
                        st_sb = st_pool.tile([128, ihw], F32R, tag="st")
                        p, r0 = divmod(h, 2)
                        r0 *= 64
                        for q in range(ihw // 512):
                            nc.tensor.matmul(
                                st_ps[:, 512 * q: 512 * (q + 1)],
                                ktp[p][r0:r0 + 64, 128 * j: 128 * (j + 1)],
                                qtp[p][r0:r0 + 64,
                                       i0 + 512 * q: i0 + 512 * (q + 1)],
                                start=True, stop=True)
                        nc.scalar.activation(
                            st_sb[:, 0:ihw], st_ps[:, 0:ihw],
                            mybir.ActivationFunctionType.Exp, scale=scale)
                        for q in range(ihw // 512):
                            nc.tensor.matmul(
                                ot[:, 512 * q: 512 * (q + 1)],
                                vaug[j][:, 65 * h: 65 * h + 65],
                                st_sb[:, 512 * q: 512 * (q + 1)],
                                start=(j == 0), stop=(j == nj - 1))
                    # normalize: rows 0..63 /= row 64
                    dsb = nrm_pool.tile([1, ihw], F32, tag="dsb")
                    nc.vector.tensor_copy(dsb[:], ot[64:65, 0:ihw])
                    rsb = nrm_pool.tile([1, ihw], F32, tag="rsb")
                    nc.vector.reciprocal_approx_fast(rsb[:], dsb[:])
                    rsr = nrm_pool.tile([1, ihw], F32R, tag="rsr")
                    nc.vector.tensor_copy(rsr[:], rsb[:])
                    bps = mm_ps.tile([128, 1024], F32, tag="mm")
                    for q in range(ihw // 512):
                        nc.tensor.matmul(
                            bps[0:64, 512 * q: 512 * (q + 1)],
                            ones1[:],
                            rsr[:, 512 * q: 512 * (q + 1)],
                            start=True, stop=True)
                    bsb = nrm_pool.tile([64, ihw], F32, tag="bsb")
                    nc.vector.tensor_copy(bsb[:], bps[0:64, 0:ihw])
                    nc.vector.tensor_mul(
                        opair[h // 2][64 * (h % 2): 64 * (h % 2) + 64,
                                      i0: i0 + ihw],
                        ot[0:64, 0:ihw], bsb[:])

        # ---- output projection partials -> DRAM ----
        with tc.tile_pool(name="pp", bufs=4) as pp_pool:
            for it in range(n // 128):
                for oc in range(2):
                    ps = mm_ps.tile([128, 1024], F32, tag="mm")
                    for p in range(2):
                        nc.tensor.matmul(
                            ps[:, 0:512],
                            opair[p][:, 128 * it: 128 * (it + 1)],
                            wot[:, 1024 * p + 512 * oc: 1024 * p + 512 * oc + 512],
                            start=(p == 0), stop=(p == 1))
                    pp_sb = pp_pool.tile([128, 512], F32, tag="pp")
                    nc.vector.tensor_copy(pp_sb[:], ps[:, 0:512])
                    nc.sync.dma_start(
                        partial_d[128 * it: 128 * (it + 1),
                                  512 * oc: 512 * oc + 512],
                        pp_sb[:])

        # ---- ReduceScatter within each batch group, then +bias ----
        nc.gpsimd.collective_compute(
            "ReduceScatter", mybir.AluOpType.add,
            replica_groups=GROUPS,
            ins=[partial_d[:]], outs=[rs_out_d[:]])

        with tc.tile_pool(name="fin", bufs=2) as fin_pool:
            for t in range(ntok_out // 128):
                rs_sb = fin_pool.tile([128, D], F32, tag="rs")
                nc.sync.dma_start(rs_sb[:], rs_out_d[128 * t: 128 * (t + 1), :])
                fo = fin_pool.tile([128, D], F32, tag="fo")
                nc.vector.tensor_add(fo[:], rs_sb[:], bias_sb[:])
                nc.sync.dma_start(out_d[128 * t: 128 * (t + 1), :], fo[:])

    nc.compile()
    return nc


def make_in_maps(x, wq, wk, wv, wo, bo):
    """Host-side sharding + layout prep (transposes/slices only)."""
    x = np.asarray(x, dtype=np.float32)
    bo_b = np.ascontiguousarray(
        np.broadcast_to(np.asarray(bo, np.float32)[None, :], (128, D)))
    in_maps = []
    for c in range(N_CORES):
        b, g = divmod(c, 4)
        r0 = CPC * g
        in_maps.append({
            "xt": np.ascontiguousarray(x[b].T),
            "wqt": np.ascontiguousarray(np.asarray(wq, np.float32)[r0:r0 + CPC, :].T),
            "wkt": np.ascontiguousarray(np.asarray(wk, np.float32)[r0:r0 + CPC, :].T),
            "wvt": np.ascontiguousarray(np.asarray(wv, np.float32)[r0:r0 + CPC, :].T),
            "wot": np.ascontiguousarray(np.asarray(wo, np.float32)[:, r0:r0 + CPC].T),
            "bob": bo_b,
        })
    return in_maps


_PROG_CACHE = {}


def _get_prog(n=N):
    if n not in _PROG_CACHE:
        _PROG_CACHE[n] = build_program(n)
    return _PROG_CACHE[n]


def run(x, wq, wk, wv, wo, bo, trace=False, trace_cores=None):
    """Run on hardware; returns (output [B,N,D], exec_time_ns or None)."""
    from concourse.bass_utils import run_bass_kernel_spmd

    nc = _get_prog()
    in_maps = make_in_maps(x, wq, wk, wv, wo, bo)
    kw = {}
    if trace:
        kw = dict(trace=True, trace_cores=trace_cores or [0])
    res = run_bass_kernel_spmd(nc, in_maps, list(range(N_CORES)), **kw)
    out = np.empty((B, N, D), dtype=np.float32)
    for c in range(N_CORES):
        b, g = divmod(c, 4)
        t0 = (N // 4) * g
        out[b, t0:t0 + N // 4, :] = res.results[c]["out"]
    return out, res.exec_time_ns


def kernel(x, wq, wk, wv, wo, bo):
    out, _ = run(x, wq, wk, wv, wo, bo)
    return out


# revision 9
# speedup vs baseline: 1.7435x; 1.7435x over previous
"""Multi-head attention (b=2, n=2048, d=1024, h=16) on 8 TRN2 NeuronCores.

Sharding: data-parallel over batch (2) x tensor-parallel over head-groups (4).
Core c handles batch c//4, heads 4*(c%4)..4*(c%4)+3 (channel rows 256*(c%4)..).
Column-parallel QKV, row-parallel output projection with on-device
ReduceScatter (bf16, overlapped chunks) over each 4-core batch group; each
core emits its token slices of the final output which the host reassembles.

Matmul operands are bf16 (PE full rate; fp32 PSUM accumulation); softmax
statistics and normalization run in fp32. Host-side prep is layout-only
(slicing/transpose/dtype): the device receives x^T and weight shards
pre-transposed so every matmul operand is already in its natural
(contraction-on-partition) layout.
"""

import sys
from contextlib import ExitStack

_TRN_REPO = "/opt/trn_rl_repo"
if _TRN_REPO not in sys.path:
    sys.path.insert(0, _TRN_REPO)

import ml_dtypes
import numpy as np

import concourse.bass as bass
import concourse.bacc as bacc
import concourse.tile as tile
from concourse import mybir

F32 = mybir.dt.float32
BF16 = mybir.dt.bfloat16

B = 2          # batch
N = 2048       # tokens
D = 1024       # model dim
H = 16         # heads
HD = D // H    # 64 head dim
N_CORES = 8
GROUPS = [[0, 1, 2, 3], [4, 5, 6, 7]]
HPC = 4        # heads per core
CPC = HPC * HD  # 256 channels per core
BW = 512       # attention i-block / RS chunk width (tokens)


def build_program(n=N):
    assert n % BW == 0
    nj = n // 128           # key tiles
    nblk = n // BW          # i blocks == RS chunks
    ntok_out = n // 4       # tokens owned per core

    nc = bacc.Bacc("TRN2", target_bir_lowering=False, debug=False,
                   num_devices=N_CORES)

    # ---- DRAM I/O (per-core shards, host-prepared, bf16) ----
    xt_d = nc.dram_tensor("xt", [D, n], BF16, kind="ExternalInput").ap()
    wqt_d = nc.dram_tensor("wqt", [D, CPC], BF16, kind="ExternalInput").ap()
    wkt_d = nc.dram_tensor("wkt", [D, CPC], BF16, kind="ExternalInput").ap()
    wvt_d = nc.dram_tensor("wvt", [D, CPC], BF16, kind="ExternalInput").ap()
    wot_d = nc.dram_tensor("wot", [CPC, D], BF16, kind="ExternalInput").ap()
    bo_d = nc.dram_tensor("bob", [128, D], F32, kind="ExternalInput").ap()
    out_d = nc.dram_tensor("out", [ntok_out, D], F32, kind="ExternalOutput").ap()

    part_d = [nc.dram_tensor(f"part{k}", [BW, D], BF16).ap() for k in range(nblk)]
    rs_d = [nc.dram_tensor(f"rsc{k}", [BW // 4, D], BF16).ap() for k in range(nblk)]

    with tile.TileContext(nc) as tc, ExitStack() as octx:
        wpool = octx.enter_context(tc.tile_pool(name="wpool", bufs=1))
        qk_pool = octx.enter_context(tc.tile_pool(name="qk", bufs=1))
        v_pool = octx.enter_context(tc.tile_pool(name="vaug", bufs=1))
        o_pool = octx.enter_context(tc.tile_pool(name="opair", bufs=1))
        xt_pool = octx.enter_context(tc.tile_pool(name="xt", bufs=1))
        st_pool = octx.enter_context(tc.tile_pool(name="stp", bufs=8))
        nrm_pool = octx.enter_context(tc.tile_pool(name="nrm", bufs=2))
        pp_pool = octx.enter_context(tc.tile_pool(name="pp", bufs=4))
        fin_pool = octx.enter_context(tc.tile_pool(name="fin", bufs=2))
        # PSUM banks: st 2x[128,1024]f32 = 4, ot 3x[65,512] = 3, mm 1x[128,512] = 1
        mm_ps = octx.enter_context(tc.tile_pool(name="mmps", bufs=1, space="PSUM"))
        st_ps_pool = octx.enter_context(
            tc.tile_pool(name="stps", bufs=2, space="PSUM"))
        ot_ps = octx.enter_context(tc.tile_pool(name="otps", bufs=3, space="PSUM"))

        # ---- weights ----
        def load_w(name, dram, rows, cols):
            nch = rows // 128
            raw = wpool.tile([128, nch * cols], BF16, tag=name, name=name + "_t")
            nc.sync.dma_start(
                raw[:].rearrange("p (c m) -> p c m", c=nch),
                dram.rearrange("(c p) m -> p c m", p=128))
            return raw[:]

        wqt = load_w("wqt", wqt_d, D, CPC)
        wkt = load_w("wkt", wkt_d, D, CPC)
        wvt = load_w("wvt", wvt_d, D, CPC)
        wot = load_w("wot", wot_d, CPC, D)

        bias_sb = wpool.tile([128, D], F32, tag="bias")
        nc.sync.dma_start(bias_sb[:], bo_d[:])

        ones_f = wpool.tile([128, 64], F32, tag="ones_f")
        nc.gpsimd.memset(ones_f[:], 1.0)
        ones1 = wpool.tile([1, 64], BF16, tag="ones1")
        nc.vector.tensor_copy(ones1[:], ones_f[0:1, :])

        qtp = [qk_pool.tile([128, n], BF16, tag=f"qtp{p}", name=f"qtp{p}")
               for p in range(2)]
        ktp = [qk_pool.tile([128, n], BF16, tag=f"ktp{p}", name=f"ktp{p}")
               for p in range(2)]
        vaug = [v_pool.tile([128, HPC * 65], BF16, tag=f"vaug{j}", name=f"vaug{j}")
                for j in range(nj)]
        opair = [o_pool.tile([128, n], BF16, tag=f"op{p}", name=f"op{p}")
                 for p in range(2)]

        # ---- x^T ----
        xt_sb = [xt_pool.tile([128, n], BF16, tag=f"xtr{ch}", name=f"xtr{ch}")
                 for ch in range(8)]
        for ch in range(8):
            nc.sync.dma_start(xt_sb[ch][:], xt_d[128 * ch:128 * (ch + 1), :])

        def qkv_pair(p):
            for (wmat, dst) in ((wqt, qtp), (wkt, ktp)):
                for ic in range(n // 512):
                    ps = mm_ps.tile([128, 512], F32, tag="mm")
                    for ch in range(8):
                        nc.tensor.matmul(
                            ps[:],
                            wmat[:, ch * 256 + p * 128: ch * 256 + p * 128 + 128],
                            xt_sb[ch][:, 512 * ic: 512 * (ic + 1)],
                            start=(ch == 0), stop=(ch == 7))
                    nc.vector.tensor_copy(
                        dst[p][:, 512 * ic: 512 * (ic + 1)], ps[:])

        def v_phase():
            for j in range(nj):
                nc.vector.tensor_copy(
                    vaug[j][:].rearrange("p (h m) -> p h m", h=HPC)[:, :, 64:65],
                    ones_f[:].rearrange("p (h m) -> p h m", m=1)[:, 0:HPC, :])
                for half in range(2):
                    ps = mm_ps.tile([128, 512], F32, tag="mm")
                    for ch in range(8):
                        nc.tensor.matmul(
                            ps[:, 0:128],
                            xt_sb[ch][:, 128 * j: 128 * (j + 1)],
                            wvt[:, ch * 256 + 128 * half:
                                ch * 256 + 128 * half + 128],
                            start=(ch == 0), stop=(ch == 7))
                    dst = vaug[j][:].rearrange(
                        "p (h m) -> p h m", h=HPC)[:, 2 * half: 2 * half + 2, 0:64]
                    src = ps[:, 0:128].rearrange("p (h m) -> p h m", h=2)
                    nc.vector.tensor_copy(dst, src)

        scale = float(HD) ** -0.5

        def attn_block(p, ib):
            """Heads 2p,2p+1 for i-block ib. Scores for both heads land in one
            [128,1024] PSUM tile (head-even cols 0-511, head-odd 512-1023) so a
            single FD=1024 exp serves both."""
            i0 = ib * BW
            ots = [ot_ps.tile([65, BW], F32, tag="ot", name=f"ot{p}_{ib}_{e}")
                   for e in range(2)]
            for j in range(nj):
                st_ps = st_ps_pool.tile([128, 1024], F32, tag="st")
                for e in range(2):
                    r0 = 64 * e
                    nc.tensor.matmul(
                        st_ps[:, 512 * e: 512 * e + 512],
                        ktp[p][r0:r0 + 64, 128 * j: 128 * (j + 1)],
                        qtp[p][r0:r0 + 64, i0: i0 + BW],
                        start=True, stop=True)
                st_sb = st_pool.tile([128, 1024], BF16, tag="st")
                nc.scalar.activation(
                    st_sb[:], st_ps[:],
                    mybir.ActivationFunctionType.Exp, scale=scale)
                for e in range(2):
                    nc.tensor.matmul(
                        ots[e][:],
                        vaug[j][:, 65 * (2 * p + e): 65 * (2 * p + e) + 65],
                        st_sb[:, 512 * e: 512 * e + 512],
                        start=(j == 0), stop=(j == nj - 1))
            for e in range(2):
                dsb = nrm_pool.tile([1, BW], F32, tag="dsb")
                nc.vector.tensor_copy(dsb[:], ots[e][64:65, :])
                rsb = nrm_pool.tile([1, BW], F32, tag="rsb")
                nc.vector.reciprocal_approx_fast(rsb[:], dsb[:])
                rsr = nrm_pool.tile([1, BW], BF16, tag="rsr")
                nc.vector.tensor_copy(rsr[:], rsb[:])
                bps = mm_ps.tile([128, 512], F32, tag="mm")
                nc.tensor.matmul(bps[0:64, :], ones1[:], rsr[:],
                                 start=True, stop=True)
                bsb = nrm_pool.tile([64, BW], F32, tag="bsb")
                nc.vector.tensor_copy(bsb[:], bps[0:64, :])
                nc.vector.tensor_mul(
                    opair[p][64 * e: 64 * e + 64, i0: i0 + BW],
                    ots[e][0:64, :], bsb[:])

        def outproj_block(k):
            for it in range(BW // 128):
                itg = k * (BW // 128) + it
                for oc in range(2):
                    ps = mm_ps.tile([128, 512], F32, tag="mm")
                    for p in range(2):
                        nc.tensor.matmul(
                            ps[:],
                            opair[p][:, 128 * itg: 128 * (itg + 1)],
                            wot[:, 1024 * p + 512 * oc: 1024 * p + 512 * oc + 512],
                            start=(p == 0), stop=(p == 1))
                    pp_sb = pp_pool.tile([128, 512], BF16, tag="pp")
                    nc.vector.tensor_copy(pp_sb[:], ps[:])
                    nc.sync.dma_start(
                        part_d[k][128 * it: 128 * (it + 1),
                                  512 * oc: 512 * oc + 512],
                        pp_sb[:])

        # ---- schedule: QKV p1 and per-block outproj/RS overlap attention ----
        qkv_pair(0)
        v_phase()
        for k in range(nblk):
            attn_block(0, k)
            if k == 0:
                qkv_pair(1)
            attn_block(1, k)
            outproj_block(k)
            nc.gpsimd.collective_compute(
                "ReduceScatter", mybir.AluOpType.add, replica_groups=GROUPS,
                ins=[part_d[k][:]], outs=[rs_d[k][:]])

        # ---- +bias, emit this core's 128-token slice per chunk ----
        for k in range(nblk):
            rs_sb = fin_pool.tile([128, D], BF16, tag="rs")
            nc.sync.dma_start(rs_sb[:], rs_d[k][:])
            fo = fin_pool.tile([128, D], F32, tag="fo")
            nc.vector.tensor_add(fo[:], rs_sb[:], bias_sb[:])
            nc.sync.dma_start(out_d[128 * k: 128 * (k + 1), :], fo[:])

    nc.compile()
    return nc


def make_in_maps(x, wq, wk, wv, wo, bo):
    """Host-side sharding + layout prep (slices/transposes/dtype only)."""
    bf = ml_dtypes.bfloat16
    x = np.asarray(x, dtype=np.float32)
    bo_b = np.ascontiguousarray(
        np.broadcast_to(np.asarray(bo, np.float32)[None, :], (128, D)))
    wq, wk, wv, wo = (np.asarray(w, np.float32) for w in (wq, wk, wv, wo))
    in_maps = []
    for c in range(N_CORES):
        b, g = divmod(c, 4)
        r0 = CPC * g
        in_maps.append({
            "xt": np.ascontiguousarray(x[b].T.astype(bf)),
            "wqt": np.ascontiguousarray(wq[r0:r0 + CPC, :].T.astype(bf)),
            "wkt": np.ascontiguousarray(wk[r0:r0 + CPC, :].T.astype(bf)),
            "wvt": np.ascontiguousarray(wv[r0:r0 + CPC, :].T.astype(bf)),
            "wot": np.ascontiguousarray(wo[:, r0:r0 + CPC].T.astype(bf)),
            "bob": bo_b,
        })
    return in_maps


_PROG_CACHE = {}


def _get_prog(n=N):
    if n not in _PROG_CACHE:
        _PROG_CACHE[n] = build_program(n)
    return _PROG_CACHE[n]


def run(x, wq, wk, wv, wo, bo, trace=False, trace_cores=None):
    """Run on hardware; returns (output [B,N,D], exec_time_ns or None)."""
    from concourse.bass_utils import run_bass_kernel_spmd

    nc = _get_prog()
    in_maps = make_in_maps(x, wq, wk, wv, wo, bo)
    kw = {}
    if trace:
        kw = dict(trace=True, trace_cores=trace_cores or [0])
    res = run_bass_kernel_spmd(nc, in_maps, list(range(N_CORES)), **kw)
    out = np.empty((B, N, D), dtype=np.float32)
    nblk = N // BW
    for c in range(N_CORES):
        b, g = divmod(c, 4)
        o = res.results[c]["out"]
        for k in range(nblk):
            t0 = BW * k + 128 * g
            out[b, t0:t0 + 128, :] = o[128 * k: 128 * (k + 1)]
    return out, res.exec_time_ns


def kernel(x, wq, wk, wv, wo, bo):
    out, _ = run(x, wq, wk, wv, wo, bo)
    return out


# revision 10
# speedup vs baseline: 1.8474x; 1.0596x over previous
"""Multi-head attention (b=2, n=2048, d=1024, h=16) on 8 TRN2 NeuronCores.

Sharding: data-parallel over batch (2) x tensor-parallel over head-groups (4).
Core c handles batch c//4, heads 4*(c%4)..4*(c%4)+3 (channel rows 256*(c%4)..).
Column-parallel QKV, row-parallel output projection with on-device
ReduceScatter (bf16, overlapped chunks) over each 4-core batch group; each
core emits its token slices of the final output which the host reassembles.

Matmul operands are bf16 (PE full rate; fp32 PSUM accumulation); softmax
statistics and normalization run in fp32. Host-side prep is layout-only
(slicing/transpose/dtype): the device receives x^T and weight shards
pre-transposed so every matmul operand is already in its natural
(contraction-on-partition) layout.
"""

import sys
from contextlib import ExitStack

_TRN_REPO = "/opt/trn_rl_repo"
if _TRN_REPO not in sys.path:
    sys.path.insert(0, _TRN_REPO)

import ml_dtypes
import numpy as np

import concourse.bass as bass
import concourse.bacc as bacc
import concourse.tile as tile
from concourse import mybir

F32 = mybir.dt.float32
BF16 = mybir.dt.bfloat16

B = 2          # batch
N = 2048       # tokens
D = 1024       # model dim
H = 16         # heads
HD = D // H    # 64 head dim
N_CORES = 8
GROUPS = [[0, 1, 2, 3], [4, 5, 6, 7]]
HPC = 4        # heads per core
CPC = HPC * HD  # 256 channels per core
BW = 512       # attention i-block / RS chunk width (tokens)


def build_program(n=N):
    assert n % BW == 0
    nj = n // 128           # key tiles
    nblk = n // BW          # i blocks == RS chunks
    ntok_out = n // 4       # tokens owned per core

    nc = bacc.Bacc("TRN2", target_bir_lowering=False, debug=False,
                   num_devices=N_CORES)

    # ---- DRAM I/O (per-core shards, host-prepared, bf16) ----
    xt_d = nc.dram_tensor("xt", [D, n], BF16, kind="ExternalInput").ap()
    wqt_d = nc.dram_tensor("wqt", [D, CPC], BF16, kind="ExternalInput").ap()
    wkt_d = nc.dram_tensor("wkt", [D, CPC], BF16, kind="ExternalInput").ap()
    wvt_d = nc.dram_tensor("wvt", [D, CPC], BF16, kind="ExternalInput").ap()
    wot_d = nc.dram_tensor("wot", [CPC, D], BF16, kind="ExternalInput").ap()
    bo_d = nc.dram_tensor("bob", [128, D], F32, kind="ExternalInput").ap()
    out_d = nc.dram_tensor("out", [ntok_out, D], F32, kind="ExternalOutput").ap()

    part_d = [nc.dram_tensor(f"part{k}", [BW, D], BF16).ap() for k in range(nblk)]
    rs_d = [nc.dram_tensor(f"rsc{k}", [BW // 4, D], BF16).ap() for k in range(nblk)]

    with tile.TileContext(nc) as tc, ExitStack() as octx:
        wpool = octx.enter_context(tc.tile_pool(name="wpool", bufs=1))
        qk_pool = octx.enter_context(tc.tile_pool(name="qk", bufs=1))
        v_pool = octx.enter_context(tc.tile_pool(name="vaug", bufs=1))
        o_pool = octx.enter_context(tc.tile_pool(name="opair", bufs=1))
        xt_pool = octx.enter_context(tc.tile_pool(name="xt", bufs=1))
        st_pool = octx.enter_context(tc.tile_pool(name="stp", bufs=8))
        nrm_pool = octx.enter_context(tc.tile_pool(name="nrm", bufs=2))
        pp_pool = octx.enter_context(tc.tile_pool(name="pp", bufs=4))
        fin_pool = octx.enter_context(tc.tile_pool(name="fin", bufs=2))
        # PSUM banks: st 2x[128,1024]f32 = 4, ot 2x[65,512] = 2, mm 2x[128,512] = 2
        mm_ps = octx.enter_context(tc.tile_pool(name="mmps", bufs=2, space="PSUM"))
        st_ps_pool = octx.enter_context(
            tc.tile_pool(name="stps", bufs=2, space="PSUM"))
        ot_ps = octx.enter_context(tc.tile_pool(name="otps", bufs=2, space="PSUM"))

        # ---- weights ----
        def load_w(name, dram, rows, cols):
            nch = rows // 128
            raw = wpool.tile([128, nch * cols], BF16, tag=name, name=name + "_t")
            nc.sync.dma_start(
                raw[:].rearrange("p (c m) -> p c m", c=nch),
                dram.rearrange("(c p) m -> p c m", p=128))
            return raw[:]

        wqt = load_w("wqt", wqt_d, D, CPC)
        wkt = load_w("wkt", wkt_d, D, CPC)
        wvt = load_w("wvt", wvt_d, D, CPC)
        wot = load_w("wot", wot_d, CPC, D)

        bias_sb = wpool.tile([128, D], F32, tag="bias")
        nc.sync.dma_start(bias_sb[:], bo_d[:])

        ones_f = wpool.tile([128, 64], F32, tag="ones_f")
        nc.gpsimd.memset(ones_f[:], 1.0)
        ones1 = wpool.tile([1, 64], BF16, tag="ones1")
        nc.vector.tensor_copy(ones1[:], ones_f[0:1, :])

        qtp = [qk_pool.tile([128, n], BF16, tag=f"qtp{p}", name=f"qtp{p}")
               for p in range(2)]
        ktp = [qk_pool.tile([128, n], BF16, tag=f"ktp{p}", name=f"ktp{p}")
               for p in range(2)]
        vaug = [v_pool.tile([128, HPC * 65], BF16, tag=f"vaug{j}", name=f"vaug{j}")
                for j in range(nj)]
        opair = [o_pool.tile([128, n], BF16, tag=f"op{p}", name=f"op{p}")
                 for p in range(2)]

        # ---- x^T ----
        xt_sb = [xt_pool.tile([128, n], BF16, tag=f"xtr{ch}", name=f"xtr{ch}")
                 for ch in range(8)]
        for ch in range(8):
            nc.sync.dma_start(xt_sb[ch][:], xt_d[128 * ch:128 * (ch + 1), :])

        def qkv_pair(p):
            for (wmat, dst) in ((wqt, qtp), (wkt, ktp)):
                for ic in range(n // 512):
                    ps = mm_ps.tile([128, 512], F32, tag="mm")
                    for ch in range(8):
                        nc.tensor.matmul(
                            ps[:],
                            wmat[:, ch * 256 + p * 128: ch * 256 + p * 128 + 128],
                            xt_sb[ch][:, 512 * ic: 512 * (ic + 1)],
                            start=(ch == 0), stop=(ch == 7))
                    nc.vector.tensor_copy(
                        dst[p][:, 512 * ic: 512 * (ic + 1)], ps[:])

        def v_phase():
            for j in range(nj):
                nc.vector.tensor_copy(
                    vaug[j][:].rearrange("p (h m) -> p h m", h=HPC)[:, :, 64:65],
                    ones_f[:].rearrange("p (h m) -> p h m", m=1)[:, 0:HPC, :])
                for half in range(2):
                    ps = mm_ps.tile([128, 512], F32, tag="mm")
                    for ch in range(8):
                        nc.tensor.matmul(
                            ps[:, 0:128],
                            xt_sb[ch][:, 128 * j: 128 * (j + 1)],
                            wvt[:, ch * 256 + 128 * half:
                                ch * 256 + 128 * half + 128],
                            start=(ch == 0), stop=(ch == 7))
                    dst = vaug[j][:].rearrange(
                        "p (h m) -> p h m", h=HPC)[:, 2 * half: 2 * half + 2, 0:64]
                    src = ps[:, 0:128].rearrange("p (h m) -> p h m", h=2)
                    nc.vector.tensor_copy(dst, src)

        scale = float(HD) ** -0.5

        def attn_block(p, ib):
            """Heads 2p,2p+1 for i-block ib. Scores for both heads land in one
            [128,1024] PSUM tile (head-even cols 0-511, head-odd 512-1023) so a
            single FD=1024 exp serves both."""
            i0 = ib * BW
            ots = [ot_ps.tile([65, BW], F32, tag="ot", name=f"ot{p}_{ib}_{e}")
                   for e in range(2)]
            def emit_av(j, st_sb):
                for e in range(2):
                    nc.tensor.matmul(
                        ots[e][:],
                        vaug[j][:, 65 * (2 * p + e): 65 * (2 * p + e) + 65],
                        st_sb[:, 512 * e: 512 * e + 512],
                        start=(j == 0), stop=(j == nj - 1))

            # AV emitted 2 iterations behind scores/exp so the in-order PE
            # never head-of-line blocks waiting for the current exp.
            pend = []
            for j in range(nj):
                st_ps = st_ps_pool.tile([128, 1024], F32, tag="st")
                for e in range(2):
                    r0 = 64 * e
                    nc.tensor.matmul(
                        st_ps[:, 512 * e: 512 * e + 512],
                        ktp[p][r0:r0 + 64, 128 * j: 128 * (j + 1)],
                        qtp[p][r0:r0 + 64, i0: i0 + BW],
                        start=True, stop=True)
                st_sb = st_pool.tile([128, 1024], BF16, tag="st")
                nc.scalar.activation(
                    st_sb[:], st_ps[:],
                    mybir.ActivationFunctionType.Exp, scale=scale)
                pend.append((j, st_sb))
                if len(pend) > 2:
                    emit_av(*pend.pop(0))
            for item in pend:
                emit_av(*item)
            for e in range(2):
                dsb = nrm_pool.tile([1, BW], F32, tag="dsb")
                nc.vector.tensor_copy(dsb[:], ots[e][64:65, :])
                rsb = nrm_pool.tile([1, BW], F32, tag="rsb")
                nc.vector.reciprocal_approx_fast(rsb[:], dsb[:])
                rsr = nrm_pool.tile([1, BW], BF16, tag="rsr")
                nc.vector.tensor_copy(rsr[:], rsb[:])
                bps = mm_ps.tile([128, 512], F32, tag="mm")
                nc.tensor.matmul(bps[0:64, :], ones1[:], rsr[:],
                                 start=True, stop=True)
                bsb = nrm_pool.tile([64, BW], F32, tag="bsb")
                nc.vector.tensor_copy(bsb[:], bps[0:64, :])
                nc.vector.tensor_mul(
                    opair[p][64 * e: 64 * e + 64, i0: i0 + BW],
                    ots[e][0:64, :], bsb[:])

        def outproj_block(k):
            for it in range(BW // 128):
                itg = k * (BW // 128) + it
                for oc in range(2):
                    ps = mm_ps.tile([128, 512], F32, tag="mm")
                    for p in range(2):
                        nc.tensor.matmul(
                            ps[:],
                            opair[p][:, 128 * itg: 128 * (itg + 1)],
                            wot[:, 1024 * p + 512 * oc: 1024 * p + 512 * oc + 512],
                            start=(p == 0), stop=(p == 1))
                    pp_sb = pp_pool.tile([128, 512], BF16, tag="pp")
                    nc.vector.tensor_copy(pp_sb[:], ps[:])
                    nc.sync.dma_start(
                        part_d[k][128 * it: 128 * (it + 1),
                                  512 * oc: 512 * oc + 512],
                        pp_sb[:])

        # ---- schedule: QKV p1 and per-block outproj/RS overlap attention ----
        qkv_pair(0)
        v_phase()
        for k in range(nblk):
            attn_block(0, k)
            if k == 0:
                qkv_pair(1)
            attn_block(1, k)
            outproj_block(k)
            nc.gpsimd.collective_compute(
                "ReduceScatter", mybir.AluOpType.add, replica_groups=GROUPS,
                ins=[part_d[k][:]], outs=[rs_d[k][:]])

        # ---- +bias, emit this core's 128-token slice per chunk ----
        for k in range(nblk):
            rs_sb = fin_pool.tile([128, D], BF16, tag="rs")
            nc.sync.dma_start(rs_sb[:], rs_d[k][:])
            fo = fin_pool.tile([128, D], F32, tag="fo")
            nc.vector.tensor_add(fo[:], rs_sb[:], bias_sb[:])
            nc.sync.dma_start(out_d[128 * k: 128 * (k + 1), :], fo[:])

    nc.compile()
    return nc


def make_in_maps(x, wq, wk, wv, wo, bo):
    """Host-side sharding + layout prep (slices/transposes/dtype only)."""
    bf = ml_dtypes.bfloat16
    x = np.asarray(x, dtype=np.float32)
    bo_b = np.ascontiguousarray(
        np.broadcast_to(np.asarray(bo, np.float32)[None, :], (128, D)))
    wq, wk, wv, wo = (np.asarray(w, np.float32) for w in (wq, wk, wv, wo))
    in_maps = []
    for c in range(N_CORES):
        b, g = divmod(c, 4)
        r0 = CPC * g
        in_maps.append({
            "xt": np.ascontiguousarray(x[b].T.astype(bf)),
            "wqt": np.ascontiguousarray(wq[r0:r0 + CPC, :].T.astype(bf)),
            "wkt": np.ascontiguousarray(wk[r0:r0 + CPC, :].T.astype(bf)),
            "wvt": np.ascontiguousarray(wv[r0:r0 + CPC, :].T.astype(bf)),
            "wot": np.ascontiguousarray(wo[:, r0:r0 + CPC].T.astype(bf)),
            "bob": bo_b,
        })
    return in_maps


_PROG_CACHE = {}


def _get_prog(n=N):
    if n not in _PROG_CACHE:
        _PROG_CACHE[n] = build_program(n)
    return _PROG_CACHE[n]


def run(x, wq, wk, wv, wo, bo, trace=False, trace_cores=None):
    """Run on hardware; returns (output [B,N,D], exec_time_ns or None)."""
    from concourse.bass_utils import run_bass_kernel_spmd

    nc = _get_prog()
    in_maps = make_in_maps(x, wq, wk, wv, wo, bo)
    kw = {}
    if trace:
        kw = dict(trace=True, trace_cores=trace_cores or [0])
    res = run_bass_kernel_spmd(nc, in_maps, list(range(N_CORES)), **kw)
    out = np.empty((B, N, D), dtype=np.float32)
    nblk = N // BW
    for c in range(N_CORES):
        b, g = divmod(c, 4)
        o = res.results[c]["out"]
        for k in range(nblk):
            t0 = BW * k + 128 * g
            out[b, t0:t0 + 128, :] = o[128 * k: 128 * (k + 1)]
    return out, res.exec_time_ns


def kernel(x, wq, wk, wv, wo, bo):
    out, _ = run(x, wq, wk, wv, wo, bo)
    return out


# revision 12
# speedup vs baseline: 1.8812x; 1.0183x over previous
"""Multi-head attention (b=2, n=2048, d=1024, h=16) on 8 TRN2 NeuronCores.

Sharding: data-parallel over batch (2) x tensor-parallel over head-groups (4).
Core c handles batch c//4, heads 4*(c%4)..4*(c%4)+3 (channel rows 256*(c%4)..).
Column-parallel QKV, row-parallel output projection with on-device
ReduceScatter (bf16, overlapped chunks) over each 4-core batch group; each
core emits its token slices of the final output which the host reassembles.

Matmul operands are bf16 (PE full rate; fp32 PSUM accumulation); softmax
statistics and normalization run in fp32. Host-side prep is layout-only
(slicing/transpose/dtype): the device receives x^T and weight shards
pre-transposed so every matmul operand is already in its natural
(contraction-on-partition) layout.
"""

import sys
from contextlib import ExitStack

_TRN_REPO = "/opt/trn_rl_repo"
if _TRN_REPO not in sys.path:
    sys.path.insert(0, _TRN_REPO)

import ml_dtypes
import numpy as np

import concourse.bass as bass
import concourse.bacc as bacc
import concourse.tile as tile
from concourse import mybir

F32 = mybir.dt.float32
BF16 = mybir.dt.bfloat16

B = 2          # batch
N = 2048       # tokens
D = 1024       # model dim
H = 16         # heads
HD = D // H    # 64 head dim
N_CORES = 8
GROUPS = [[0, 1, 2, 3], [4, 5, 6, 7]]
HPC = 4        # heads per core
CPC = HPC * HD  # 256 channels per core
BW = 512       # attention i-block / RS chunk width (tokens)


def build_program(n=N):
    assert n % BW == 0
    nj = n // 128           # key tiles
    nblk = n // BW          # i blocks == RS chunks
    ntok_out = n // 4       # tokens owned per core

    nc = bacc.Bacc("TRN2", target_bir_lowering=False, debug=False,
                   num_devices=N_CORES)

    # ---- DRAM I/O (per-core shards, host-prepared, bf16) ----
    xt_d = nc.dram_tensor("xt", [D, n], BF16, kind="ExternalInput").ap()
    wqt_d = nc.dram_tensor("wqt", [D, CPC], BF16, kind="ExternalInput").ap()
    wkt_d = nc.dram_tensor("wkt", [D, CPC], BF16, kind="ExternalInput").ap()
    wvt_d = nc.dram_tensor("wvt", [D, CPC], BF16, kind="ExternalInput").ap()
    wot_d = nc.dram_tensor("wot", [CPC, D], BF16, kind="ExternalInput").ap()
    bo_d = nc.dram_tensor("bob", [128, D], F32, kind="ExternalInput").ap()
    out_d = nc.dram_tensor("out", [ntok_out, D], F32, kind="ExternalOutput").ap()

    part_d = [nc.dram_tensor(f"part{k}", [BW, D], BF16).ap() for k in range(nblk)]
    rs_d = [nc.dram_tensor(f"rsc{k}", [BW // 4, D], BF16).ap() for k in range(nblk)]

    with tile.TileContext(nc) as tc, ExitStack() as octx:
        wpool = octx.enter_context(tc.tile_pool(name="wpool", bufs=1))
        qk_pool = octx.enter_context(tc.tile_pool(name="qk", bufs=1))
        v_pool = octx.enter_context(tc.tile_pool(name="vaug", bufs=1))
        o_pool = octx.enter_context(tc.tile_pool(name="opair", bufs=1))
        xt_pool = octx.enter_context(tc.tile_pool(name="xt", bufs=1))
        st_pool = octx.enter_context(tc.tile_pool(name="stp", bufs=8))
        nrm_pool = octx.enter_context(tc.tile_pool(name="nrm", bufs=2))
        pp_pool = octx.enter_context(tc.tile_pool(name="pp", bufs=4))
        fin_pool = octx.enter_context(tc.tile_pool(name="fin", bufs=2))
        # PSUM banks: st 2x[128,1024]f32 = 4, ot 2x[65,512] = 2, mm 2x[128,512] = 2
        mm_ps = octx.enter_context(tc.tile_pool(name="mmps", bufs=2, space="PSUM"))
        st_ps_pool = octx.enter_context(
            tc.tile_pool(name="stps", bufs=2, space="PSUM"))
        ot_ps = octx.enter_context(tc.tile_pool(name="otps", bufs=2, space="PSUM"))

        # ---- weights ----
        def load_w(name, dram, rows, cols):
            nch = rows // 128
            raw = wpool.tile([128, nch * cols], BF16, tag=name, name=name + "_t")
            nc.sync.dma_start(
                raw[:].rearrange("p (c m) -> p c m", c=nch),
                dram.rearrange("(c p) m -> p c m", p=128))
            return raw[:]

        wqt = load_w("wqt", wqt_d, D, CPC)
        wkt = load_w("wkt", wkt_d, D, CPC)
        wvt = load_w("wvt", wvt_d, D, CPC)
        wot = load_w("wot", wot_d, CPC, D)

        bias_sb = wpool.tile([128, D], F32, tag="bias")
        nc.sync.dma_start(bias_sb[:], bo_d[:])

        ones_f = wpool.tile([128, 64], F32, tag="ones_f")
        nc.gpsimd.memset(ones_f[:], 1.0)
        ones1 = wpool.tile([1, 64], BF16, tag="ones1")
        nc.vector.tensor_copy(ones1[:], ones_f[0:1, :])

        qtp = [qk_pool.tile([128, n], BF16, tag=f"qtp{p}", name=f"qtp{p}")
               for p in range(2)]
        ktp = [qk_pool.tile([128, n], BF16, tag=f"ktp{p}", name=f"ktp{p}")
               for p in range(2)]
        vaug = [v_pool.tile([128, HPC * 65], BF16, tag=f"vaug{j}", name=f"vaug{j}")
                for j in range(nj)]
        opair = [o_pool.tile([128, n], BF16, tag=f"op{p}", name=f"op{p}")
                 for p in range(2)]

        # ---- x^T ----
        xt_sb = [xt_pool.tile([128, n], BF16, tag=f"xtr{ch}", name=f"xtr{ch}")
                 for ch in range(8)]
        for ch in range(8):
            nc.sync.dma_start(xt_sb[ch][:], xt_d[128 * ch:128 * (ch + 1), :])

        def qkv_pair(p):
            for (wmat, dst) in ((wqt, qtp), (wkt, ktp)):
                for ic in range(n // 512):
                    ps = mm_ps.tile([128, 512], F32, tag="mm")
                    for ch in range(8):
                        nc.tensor.matmul(
                            ps[:],
                            wmat[:, ch * 256 + p * 128: ch * 256 + p * 128 + 128],
                            xt_sb[ch][:, 512 * ic: 512 * (ic + 1)],
                            start=(ch == 0), stop=(ch == 7))
                    nc.vector.tensor_copy(
                        dst[p][:, 512 * ic: 512 * (ic + 1)], ps[:])

        def v_phase():
            for j in range(nj):
                nc.vector.tensor_copy(
                    vaug[j][:].rearrange("p (h m) -> p h m", h=HPC)[:, :, 64:65],
                    ones_f[:].rearrange("p (h m) -> p h m", m=1)[:, 0:HPC, :])
                for half in range(2):
                    ps = mm_ps.tile([128, 512], F32, tag="mm")
                    for ch in range(8):
                        nc.tensor.matmul(
                            ps[:, 0:128],
                            xt_sb[ch][:, 128 * j: 128 * (j + 1)],
                            wvt[:, ch * 256 + 128 * half:
                                ch * 256 + 128 * half + 128],
                            start=(ch == 0), stop=(ch == 7))
                    dst = vaug[j][:].rearrange(
                        "p (h m) -> p h m", h=HPC)[:, 2 * half: 2 * half + 2, 0:64]
                    src = ps[:, 0:128].rearrange("p (h m) -> p h m", h=2)
                    nc.vector.tensor_copy(dst, src)

        scale = float(HD) ** -0.5

        def attn_block(p, ib):
            """Heads 2p,2p+1 for i-block ib. Scores for both heads land in one
            [128,1024] PSUM tile (head-even cols 0-511, head-odd 512-1023) so a
            single FD=1024 exp serves both."""
            i0 = ib * BW
            ots = [ot_ps.tile([65, BW], F32, tag="ot", name=f"ot{p}_{ib}_{e}")
                   for e in range(2)]
            def emit_av(j, st_sb):
                for e in range(2):
                    nc.tensor.matmul(
                        ots[e][:],
                        vaug[j][:, 65 * (2 * p + e): 65 * (2 * p + e) + 65],
                        st_sb[:, 512 * e: 512 * e + 512],
                        start=(j == 0), stop=(j == nj - 1))

            # AV emitted 2 iterations behind scores/exp so the in-order PE
            # never head-of-line blocks waiting for the current exp.
            pend = []
            for j in range(nj):
                st_ps = st_ps_pool.tile([128, 1024], F32, tag="st")
                for e in range(2):
                    r0 = 64 * e
                    nc.tensor.matmul(
                        st_ps[:, 512 * e: 512 * e + 512],
                        ktp[p][r0:r0 + 64, 128 * j: 128 * (j + 1)],
                        qtp[p][r0:r0 + 64, i0: i0 + BW],
                        start=True, stop=True)
                st_sb = st_pool.tile([128, 1024], BF16, tag="st")
                nc.scalar.activation(
                    st_sb[:], st_ps[:],
                    mybir.ActivationFunctionType.Exp, scale=scale)
                pend.append((j, st_sb))
                if len(pend) > 2:
                    emit_av(*pend.pop(0))
            for item in pend:
                emit_av(*item)
            for e in range(2):
                dsb = nrm_pool.tile([1, BW], F32, tag="dsb")
                nc.vector.tensor_copy(dsb[:], ots[e][64:65, :])
                rsb = nrm_pool.tile([1, BW], F32, tag="rsb")
                nc.vector.reciprocal_approx_fast(rsb[:], dsb[:])
                rsr = nrm_pool.tile([1, BW], BF16, tag="rsr")
                nc.vector.tensor_copy(rsr[:], rsb[:])
                bps = mm_ps.tile([128, 512], F32, tag="mm")
                nc.tensor.matmul(bps[0:64, :], ones1[:], rsr[:],
                                 start=True, stop=True)
                bsb = nrm_pool.tile([64, BW], F32, tag="bsb")
                nc.vector.tensor_copy(bsb[:], bps[0:64, :])
                nc.vector.tensor_mul(
                    opair[p][64 * e: 64 * e + 64, i0: i0 + BW],
                    ots[e][0:64, :], bsb[:])

        def outproj_block(k):
            for it in range(BW // 128):
                itg = k * (BW // 128) + it
                for oc in range(2):
                    ps = mm_ps.tile([128, 512], F32, tag="mm")
                    for p in range(2):
                        nc.tensor.matmul(
                            ps[:],
                            opair[p][:, 128 * itg: 128 * (itg + 1)],
                            wot[:, 1024 * p + 512 * oc: 1024 * p + 512 * oc + 512],
                            start=(p == 0), stop=(p == 1))
                    pp_sb = pp_pool.tile([128, 512], BF16, tag="pp")
                    nc.vector.tensor_copy(pp_sb[:], ps[:])
                    nc.sync.dma_start(
                        part_d[k][128 * it: 128 * (it + 1),
                                  512 * oc: 512 * oc + 512],
                        pp_sb[:])

        # ---- schedule: QKV p1 and per-block outproj/RS overlap attention ----
        qkv_pair(0)
        v_phase()
        for k in range(nblk):
            attn_block(0, k)
            if k == 0:
                qkv_pair(1)
            attn_block(1, k)
            outproj_block(k)
            nc.gpsimd.collective_compute(
                "ReduceScatter", mybir.AluOpType.add, replica_groups=GROUPS,
                ins=[part_d[k][:]], outs=[rs_d[k][:]])

        # ---- +bias on idle GPSIMD (keeps the tail off the busy in-order
        #      DVE queue), emit this core's 128-token slice per chunk ----
        for k in range(nblk):
            rs_sb = fin_pool.tile([128, D], BF16, tag="rs")
            nc.sync.dma_start(rs_sb[:], rs_d[k][:])
            fo = fin_pool.tile([128, D], F32, tag="fo")
            nc.gpsimd.tensor_add(fo[:], rs_sb[:], bias_sb[:])
            nc.sync.dma_start(out_d[128 * k: 128 * (k + 1), :], fo[:])

    nc.compile()
    return nc


def make_in_maps(x, wq, wk, wv, wo, bo):
    """Host-side sharding + layout prep (slices/transposes/dtype only)."""
    bf = ml_dtypes.bfloat16
    x = np.asarray(x, dtype=np.float32)
    bo_b = np.ascontiguousarray(
        np.broadcast_to(np.asarray(bo, np.float32)[None, :], (128, D)))
    wq, wk, wv, wo = (np.asarray(w, np.float32) for w in (wq, wk, wv, wo))
    in_maps = []
    for c in range(N_CORES):
        b, g = divmod(c, 4)
        r0 = CPC * g
        in_maps.append({
            "xt": np.ascontiguousarray(x[b].T.astype(bf)),
            "wqt": np.ascontiguousarray(wq[r0:r0 + CPC, :].T.astype(bf)),
            "wkt": np.ascontiguousarray(wk[r0:r0 + CPC, :].T.astype(bf)),
            "wvt": np.ascontiguousarray(wv[r0:r0 + CPC, :].T.astype(bf)),
            "wot": np.ascontiguousarray(wo[:, r0:r0 + CPC].T.astype(bf)),
            "bob": bo_b,
        })
    return in_maps


_PROG_CACHE = {}


def _get_prog(n=N):
    if n not in _PROG_CACHE:
        _PROG_CACHE[n] = build_program(n)
    return _PROG_CACHE[n]


def run(x, wq, wk, wv, wo, bo, trace=False, trace_cores=None):
    """Run on hardware; returns (output [B,N,D], exec_time_ns or None)."""
    from concourse.bass_utils import run_bass_kernel_spmd

    nc = _get_prog()
    in_maps = make_in_maps(x, wq, wk, wv, wo, bo)
    kw = {}
    if trace:
        kw = dict(trace=True, trace_cores=trace_cores or [0])
    res = run_bass_kernel_spmd(nc, in_maps, list(range(N_CORES)), **kw)
    out = np.empty((B, N, D), dtype=np.float32)
    nblk = N // BW
    for c in range(N_CORES):
        b, g = divmod(c, 4)
        o = res.results[c]["out"]
        for k in range(nblk):
            t0 = BW * k + 128 * g
            out[b, t0:t0 + 128, :] = o[128 * k: 128 * (k + 1)]
    return out, res.exec_time_ns


def kernel(x, wq, wk, wv, wo, bo):
    out, _ = run(x, wq, wk, wv, wo, bo)
    return out


# revision 13
# speedup vs baseline: 1.9197x; 1.0205x over previous
"""Multi-head attention (b=2, n=2048, d=1024, h=16) on 8 TRN2 NeuronCores.

Sharding: data-parallel over batch (2) x tensor-parallel over head-groups (4).
Core c handles batch c//4, heads 4*(c%4)..4*(c%4)+3 (channel rows 256*(c%4)..).
Column-parallel QKV, row-parallel output projection with on-device
ReduceScatter (bf16, overlapped chunks) over each 4-core batch group; each
core emits its token slices of the final output which the host reassembles.

Matmul operands are bf16 (PE full rate; fp32 PSUM accumulation); softmax
statistics and normalization run in fp32. Host-side prep is layout-only
(slicing/transpose/dtype): the device receives x^T and weight shards
pre-transposed so every matmul operand is already in its natural
(contraction-on-partition) layout.
"""

import sys
from contextlib import ExitStack

_TRN_REPO = "/opt/trn_rl_repo"
if _TRN_REPO not in sys.path:
    sys.path.insert(0, _TRN_REPO)

import ml_dtypes
import numpy as np

import concourse.bass as bass
import concourse.bacc as bacc
import concourse.tile as tile
from concourse import mybir

F32 = mybir.dt.float32
BF16 = mybir.dt.bfloat16

B = 2          # batch
N = 2048       # tokens
D = 1024       # model dim
H = 16         # heads
HD = D // H    # 64 head dim
N_CORES = 8
GROUPS = [[0, 1, 2, 3], [4, 5, 6, 7]]
HPC = 4        # heads per core
CPC = HPC * HD  # 256 channels per core
BW = 512       # attention i-block / RS chunk width (tokens)


def build_program(n=N):
    assert n % BW == 0
    nj = n // 128           # key tiles
    nblk = n // BW          # i blocks == RS chunks
    ntok_out = n // 4       # tokens owned per core

    nc = bacc.Bacc("TRN2", target_bir_lowering=False, debug=False,
                   num_devices=N_CORES)

    # ---- DRAM I/O (per-core shards, host-prepared, bf16) ----
    xt_d = nc.dram_tensor("xt", [D, n], BF16, kind="ExternalInput").ap()
    wqt_d = nc.dram_tensor("wqt", [D, CPC], BF16, kind="ExternalInput").ap()
    wkt_d = nc.dram_tensor("wkt", [D, CPC], BF16, kind="ExternalInput").ap()
    wvt_d = nc.dram_tensor("wvt", [D, CPC], BF16, kind="ExternalInput").ap()
    wot_d = nc.dram_tensor("wot", [CPC, D], BF16, kind="ExternalInput").ap()
    bo_d = nc.dram_tensor("bob", [128, D], F32, kind="ExternalInput").ap()
    out_d = nc.dram_tensor("out", [ntok_out, D], F32, kind="ExternalOutput").ap()

    part_d = [nc.dram_tensor(f"part{k}", [BW, D], BF16).ap() for k in range(nblk)]
    rs_d = [nc.dram_tensor(f"rsc{k}", [BW // 4, D], BF16).ap() for k in range(nblk)]

    with tile.TileContext(nc) as tc, ExitStack() as octx:
        wpool = octx.enter_context(tc.tile_pool(name="wpool", bufs=1))
        qk_pool = octx.enter_context(tc.tile_pool(name="qk", bufs=1))
        v_pool = octx.enter_context(tc.tile_pool(name="vaug", bufs=1))
        o_pool = octx.enter_context(tc.tile_pool(name="opair", bufs=1))
        xt_pool = octx.enter_context(tc.tile_pool(name="xt", bufs=1))
        st_pool = octx.enter_context(tc.tile_pool(name="stp", bufs=8))
        nrm_pool = octx.enter_context(tc.tile_pool(name="nrm", bufs=2))
        pp_pool = octx.enter_context(tc.tile_pool(name="pp", bufs=8))
        fin_pool = octx.enter_context(tc.tile_pool(name="fin", bufs=2))
        # PSUM banks: st 2x[128,1024]f32 = 4, ot 2x[65,512] = 2, mm 2x[128,512] = 2
        mm_ps = octx.enter_context(tc.tile_pool(name="mmps", bufs=2, space="PSUM"))
        st_ps_pool = octx.enter_context(
            tc.tile_pool(name="stps", bufs=2, space="PSUM"))
        ot_ps = octx.enter_context(tc.tile_pool(name="otps", bufs=2, space="PSUM"))

        # ---- weights ----
        def load_w(name, dram, rows, cols):
            nch = rows // 128
            raw = wpool.tile([128, nch * cols], BF16, tag=name, name=name + "_t")
            nc.sync.dma_start(
                raw[:].rearrange("p (c m) -> p c m", c=nch),
                dram.rearrange("(c p) m -> p c m", p=128))
            return raw[:]

        wqt = load_w("wqt", wqt_d, D, CPC)
        wkt = load_w("wkt", wkt_d, D, CPC)
        wvt = load_w("wvt", wvt_d, D, CPC)
        wot = load_w("wot", wot_d, CPC, D)

        bias_sb = wpool.tile([128, D], F32, tag="bias")
        nc.sync.dma_start(bias_sb[:], bo_d[:])

        ones_f = wpool.tile([128, 64], F32, tag="ones_f")
        nc.gpsimd.memset(ones_f[:], 1.0)
        ones1 = wpool.tile([1, 64], BF16, tag="ones1")
        nc.vector.tensor_copy(ones1[:], ones_f[0:1, :])

        qtp = [qk_pool.tile([128, n], BF16, tag=f"qtp{p}", name=f"qtp{p}")
               for p in range(2)]
        ktp = [qk_pool.tile([128, n], BF16, tag=f"ktp{p}", name=f"ktp{p}")
               for p in range(2)]
        vaug = [v_pool.tile([128, HPC * 65], BF16, tag=f"vaug{j}", name=f"vaug{j}")
                for j in range(nj)]
        opair = [o_pool.tile([128, n], BF16, tag=f"op{p}", name=f"op{p}")
                 for p in range(2)]

        # ---- x^T ----
        xt_sb = [xt_pool.tile([128, n], BF16, tag=f"xtr{ch}", name=f"xtr{ch}")
                 for ch in range(8)]
        for ch in range(8):
            nc.sync.dma_start(xt_sb[ch][:], xt_d[128 * ch:128 * (ch + 1), :])

        def qkv_pair(p):
            for (wmat, dst) in ((wqt, qtp), (wkt, ktp)):
                for ic in range(n // 512):
                    ps = mm_ps.tile([128, 512], F32, tag="mm")
                    for ch in range(8):
                        nc.tensor.matmul(
                            ps[:],
                            wmat[:, ch * 256 + p * 128: ch * 256 + p * 128 + 128],
                            xt_sb[ch][:, 512 * ic: 512 * (ic + 1)],
                            start=(ch == 0), stop=(ch == 7))
                    nc.vector.tensor_copy(
                        dst[p][:, 512 * ic: 512 * (ic + 1)], ps[:])

        def v_phase():
            for j in range(nj):
                nc.vector.tensor_copy(
                    vaug[j][:].rearrange("p (h m) -> p h m", h=HPC)[:, :, 64:65],
                    ones_f[:].rearrange("p (h m) -> p h m", m=1)[:, 0:HPC, :])
                for half in range(2):
                    ps = mm_ps.tile([128, 512], F32, tag="mm")
                    for ch in range(8):
                        nc.tensor.matmul(
                            ps[:, 0:128],
                            xt_sb[ch][:, 128 * j: 128 * (j + 1)],
                            wvt[:, ch * 256 + 128 * half:
                                ch * 256 + 128 * half + 128],
                            start=(ch == 0), stop=(ch == 7))
                    dst = vaug[j][:].rearrange(
                        "p (h m) -> p h m", h=HPC)[:, 2 * half: 2 * half + 2, 0:64]
                    src = ps[:, 0:128].rearrange("p (h m) -> p h m", h=2)
                    nc.vector.tensor_copy(dst, src)

        scale = float(HD) ** -0.5

        def attn_block(p, ib):
            """Heads 2p,2p+1 for i-block ib. Scores for both heads land in one
            [128,1024] PSUM tile (head-even cols 0-511, head-odd 512-1023) so a
            single FD=1024 exp serves both."""
            i0 = ib * BW
            ots = [ot_ps.tile([65, BW], F32, tag="ot", name=f"ot{p}_{ib}_{e}")
                   for e in range(2)]
            def emit_av(j, st_sb):
                for e in range(2):
                    nc.tensor.matmul(
                        ots[e][:],
                        vaug[j][:, 65 * (2 * p + e): 65 * (2 * p + e) + 65],
                        st_sb[:, 512 * e: 512 * e + 512],
                        start=(j == 0), stop=(j == nj - 1))

            # AV emitted 2 iterations behind scores/exp so the in-order PE
            # never head-of-line blocks waiting for the current exp.
            pend = []
            for j in range(nj):
                st_ps = st_ps_pool.tile([128, 1024], F32, tag="st")
                for e in range(2):
                    r0 = 64 * e
                    nc.tensor.matmul(
                        st_ps[:, 512 * e: 512 * e + 512],
                        ktp[p][r0:r0 + 64, 128 * j: 128 * (j + 1)],
                        qtp[p][r0:r0 + 64, i0: i0 + BW],
                        start=True, stop=True)
                st_sb = st_pool.tile([128, 1024], BF16, tag="st")
                nc.scalar.activation(
                    st_sb[:], st_ps[:],
                    mybir.ActivationFunctionType.Exp, scale=scale)
                pend.append((j, st_sb))
                if len(pend) > 2:
                    emit_av(*pend.pop(0))
            for item in pend:
                emit_av(*item)
            for e in range(2):
                dsb = nrm_pool.tile([1, BW], F32, tag="dsb")
                nc.vector.tensor_copy(dsb[:], ots[e][64:65, :])
                rsb = nrm_pool.tile([1, BW], F32, tag="rsb")
                nc.vector.reciprocal_approx_fast(rsb[:], dsb[:])
                rsr = nrm_pool.tile([1, BW], BF16, tag="rsr")
                nc.vector.tensor_copy(rsr[:], rsb[:])
                bps = mm_ps.tile([128, 512], F32, tag="mm")
                nc.tensor.matmul(bps[0:64, :], ones1[:], rsr[:],
                                 start=True, stop=True)
                bsb = nrm_pool.tile([64, BW], F32, tag="bsb")
                nc.vector.tensor_copy(bsb[:], bps[0:64, :])
                nc.vector.tensor_mul(
                    opair[p][64 * e: 64 * e + 64, i0: i0 + BW],
                    ots[e][0:64, :], bsb[:])

        def outproj_block(k):
            for it in range(BW // 128):
                itg = k * (BW // 128) + it
                for oc in range(2):
                    ps = mm_ps.tile([128, 512], F32, tag="mm")
                    for p in range(2):
                        nc.tensor.matmul(
                            ps[:],
                            opair[p][:, 128 * itg: 128 * (itg + 1)],
                            wot[:, 1024 * p + 512 * oc: 1024 * p + 512 * oc + 512],
                            start=(p == 0), stop=(p == 1))
                    pp_sb = pp_pool.tile([128, 512], BF16, tag="pp")
                    nc.vector.tensor_copy(pp_sb[:], ps[:])
                    nc.sync.dma_start(
                        part_d[k][128 * it: 128 * (it + 1),
                                  512 * oc: 512 * oc + 512],
                        pp_sb[:])

        # ---- schedule: QKV p1 and per-block outproj/RS overlap attention ----
        qkv_pair(0)
        v_phase()
        for k in range(nblk):
            attn_block(0, k)
            if k == 0:
                qkv_pair(1)
            attn_block(1, k)
            outproj_block(k)
            nc.gpsimd.collective_compute(
                "ReduceScatter", mybir.AluOpType.add, replica_groups=GROUPS,
                ins=[part_d[k][:]], outs=[rs_d[k][:]])

        # ---- +bias on idle GPSIMD (keeps the tail off the busy in-order
        #      DVE queue), emit this core's 128-token slice per chunk ----
        for k in range(nblk):
            rs_sb = fin_pool.tile([128, D], BF16, tag="rs")
            nc.gpsimd.dma_start(rs_sb[:], rs_d[k][:])
            fo = fin_pool.tile([128, D], F32, tag="fo")
            nc.gpsimd.tensor_add(fo[:], rs_sb[:], bias_sb[:])
            nc.gpsimd.dma_start(out_d[128 * k: 128 * (k + 1), :], fo[:])

    nc.compile()
    return nc


def make_in_maps(x, wq, wk, wv, wo, bo):
    """Host-side sharding + layout prep (slices/transposes/dtype only)."""
    bf = ml_dtypes.bfloat16
    x = np.asarray(x, dtype=np.float32)
    bo_b = np.ascontiguousarray(
        np.broadcast_to(np.asarray(bo, np.float32)[None, :], (128, D)))
    wq, wk, wv, wo = (np.asarray(w, np.float32) for w in (wq, wk, wv, wo))
    in_maps = []
    for c in range(N_CORES):
        b, g = divmod(c, 4)
        r0 = CPC * g
        in_maps.append({
            "xt": np.ascontiguousarray(x[b].T.astype(bf)),
            "wqt": np.ascontiguousarray(wq[r0:r0 + CPC, :].T.astype(bf)),
            "wkt": np.ascontiguousarray(wk[r0:r0 + CPC, :].T.astype(bf)),
            "wvt": np.ascontiguousarray(wv[r0:r0 + CPC, :].T.astype(bf)),
            "wot": np.ascontiguousarray(wo[:, r0:r0 + CPC].T.astype(bf)),
            "bob": bo_b,
        })
    return in_maps


_PROG_CACHE = {}


def _get_prog(n=N):
    if n not in _PROG_CACHE:
        _PROG_CACHE[n] = build_program(n)
    return _PROG_CACHE[n]


def run(x, wq, wk, wv, wo, bo, trace=False, trace_cores=None):
    """Run on hardware; returns (output [B,N,D], exec_time_ns or None)."""
    from concourse.bass_utils import run_bass_kernel_spmd

    nc = _get_prog()
    in_maps = make_in_maps(x, wq, wk, wv, wo, bo)
    kw = {}
    if trace:
        kw = dict(trace=True, trace_cores=trace_cores or [0])
    res = run_bass_kernel_spmd(nc, in_maps, list(range(N_CORES)), **kw)
    out = np.empty((B, N, D), dtype=np.float32)
    nblk = N // BW
    for c in range(N_CORES):
        b, g = divmod(c, 4)
        o = res.results[c]["out"]
        for k in range(nblk):
            t0 = BW * k + 128 * g
            out[b, t0:t0 + 128, :] = o[128 * k: 128 * (k + 1)]
    return out, res.exec_time_ns


def kernel(x, wq, wk, wv, wo, bo):
    out, _ = run(x, wq, wk, wv, wo, bo)
    return out
